# revision 4
# baseline (speedup 1.0000x reference)
"""Correlation cost-volume (SpatialCorrelationSampler k=1, patch=9) + leaky ReLU.

Full inputs: feat1, feat2 [16, 256, 96, 160] f32.  Output [16, 81, 96, 160] f32.
corr[b, 9*i+j, y, x] = leaky_relu(sum_c f1[b,c,y,x] * f2[b,c,y+i-4,x+j-4], 0.1)

Strategy (8 NeuronCores, data-parallel over batch, 2 images/core):
  - host pre-casts both features to bf16 (the matmuls run in bf16 anyway)
    and pre-pads feat2 to [C, 104, 168] so every SBUF load is one large
    contiguous DMA -> input HBM traffic is halved vs f32.
  - per (image, 80-col half, row y): Gram-band matmuls on TensorE:
      lhsT = f1[c_chunk, y, x0:x0+80]            [K=128, M=80]
      rhs  = f2pad[c_chunk, y..y+8, x0..x0+87]   [K=128, 9*88] via 3D AP
    2 C-chunks accumulate in PSUM (dys 0..4 -> psum[:,0:440], dys 5..8 ->
    [512:864], each region inside one PSUM bank).
  - eviction applies leaky-relu and writes bf16 into *interleaved* S with
    col = 72*xw + 9*r + dy (r = row within an 8-row block).  VectorE
    (custom one-pass lrelu) takes dys 0..4, ScalarE (Prelu) takes dys 5..8
    so the two engines balance.  The 648 band values of partition m are
    then the contiguous run S[m, 72m : 72m+648].
  - one skewed DMA per 8-row block (partition step = rowlen+72) writes the
    band straight to DRAM as bf16; host reorders channels and upcasts.
"""

import numpy as np
import ml_dtypes

import bass_rust
import concourse.bacc as bacc
import concourse.bass as bass
import concourse.mybir as mybir
import concourse.tile as tile
from concourse.bass_utils import run_bass_kernel_spmd

B, C, H, W = 16, 256, 96, 160
NCORES = 8
NB = B // NCORES          # images per core
WH = 80                   # column-half width (matmul M)
WPAD = WH + 8             # rhs window width
NPATCH = 81
HPAD = H + 8              # host-padded feat2 rows (104)
W2 = W + 8                # host-padded feat2 cols (168)
HU = H // 2               # rows per vertical-half unit (48)
HU2 = HU + 8              # padded rows held per unit (56)
YB = 8                    # rows per S block / band DMA
SW = 9 * YB               # interleave stride per xw column (72)

BF16 = mybir.dt.bfloat16


_LRELU_OP = None


def _get_lrelu_op():
    """Register a one-pass leaky-relu custom DVE op: out = max(x, x*imm2)."""
    global _LRELU_OP
    if _LRELU_OP is not None:
        return _LRELU_OP
    from concourse import dve_ops as dops
    from concourse.dve_spec import Spec, Src0, C2, maxx, lower
    from concourse.dve_uop import DveOpSpec
    name = "LRELU_ANT"
    if name in dops.CUSTOM_DVE_SPECS:
        _LRELU_OP = next(o for o in dops.OPS if o.name == name)
        return _LRELU_OP
    spec = Spec(
        body=maxx(Src0, Src0 * C2),
        reference=lambda in0, in1, c0, c1, c2: np.maximum(in0, in0 * c2))
    opcode = dops._CUSTOM_DVE_ROW_BASE + len(dops.OPS)
    shas = {}
    for ver in ("v3", "v4"):
        try:
            o = DveOpSpec(name=name, opcode=opcode, uops=lower(spec, ver=ver),
                          rd1_en=False)
            shas[ver] = o.sha(ver)
        except Exception:
            pass
    op = dops.DveOp(name, spec, subdim=False, uops_sha=shas)
    dops.OPS.append(op)
    dops.CUSTOM_DVE_SPECS[name] = spec
    dops._SUB_OPCODE_FOR_NAME[name] = opcode
    _LRELU_OP = op
    return op


def build_nc(leaky: bool = True, units: list | None = None,
             s_bufs: int = 3) -> bass.Bass:
    lrelu_op = _get_lrelu_op()
    nc = bacc.Bacc()
    f1_ext = nc.declare_dram_parameter(
        "feat1", [NB, C, H, W], BF16, isOutput=False)
    f2_ext = nc.declare_dram_parameter(
        "feat2", [NB, C, HPAD, W2], BF16, isOutput=False)
    out_ext = nc.declare_dram_parameter(
        "out", [NB, 2, 2, HU // YB, WH, YB * NPATCH], BF16, isOutput=True)

    act_fn = (mybir.ActivationFunctionType.Prelu if leaky
              else mybir.ActivationFunctionType.Relu)
    neg = 0.1 if leaky else 0.0
    if units is None:
        units = [(b, v) for b in range(NB) for v in range(2)]

    with tile.TileContext(nc) as tc:
        with (
            tc.tile_pool(name="feat", bufs=2) as featp,
            tc.tile_pool(name="spool", bufs=s_bufs) as spool,
            tc.tile_pool(name="psum", bufs=4, space="PSUM") as psump,
        ):
            for (b, v) in units:
                y0 = HU * v
                f1u = [featp.tile([128, HU * W], BF16,
                                  tag=f"f1u{c}", name=f"f1u{c}_{b}_{v}")
                       for c in range(2)]
                f2u = [featp.tile([128, HU2 * W2], BF16,
                                  tag=f"f2u{c}", name=f"f2u{c}_{b}_{v}")
                       for c in range(2)]
                for c in range(2):
                    nc.sync.dma_start(
                        f2u[c][:, :],
                        f2_ext[b, 128 * c:128 * (c + 1), y0:y0 + HU2, :])
                    nc.sync.dma_start(
                        f1u[c][:, :],
                        f1_ext[b, 128 * c:128 * (c + 1), y0:y0 + HU, :])

                for h in range(2):
                    x0 = WH * h
                    for yb in range(HU // YB):
                        S = spool.tile([WH, SW * WPAD], BF16, tag="S",
                                       name=f"S_{b}_{v}_{h}_{yb}")
                        Sb = S[:, :]
                        RL = Sb.ap[0][0]
                        for r in range(YB):
                            yy = YB * yb + r
                            ps = psump.tile([WH, 1024], mybir.dt.float32,
                                            tag="ps",
                                            name=f"ps_{b}_{v}_{h}_{yy}")
                            for c in range(2):
                                f2b_ = f2u[c][:, :]
                                F2RL = f2b_.ap[0][0]
                                lhs = f1u[c][:, yy * W + x0:
                                             yy * W + x0 + WH]
                                rhs1 = bass_rust.AP(
                                    f2b_.tensor, f2b_.offset + yy * W2 + x0,
                                    [[F2RL, 128], [W2, 5], [1, WPAD]])
                                rhs2 = bass_rust.AP(
                                    f2b_.tensor,
                                    f2b_.offset + (yy + 5) * W2 + x0,
                                    [[F2RL, 128], [W2, 4], [1, WPAD]])
                                nc.tensor.matmul(
                                    ps[0:WH, 0:440], lhs, rhs1,
                                    start=(c == 0), stop=(c == 1))
                                nc.tensor.matmul(
                                    ps[0:WH, 512:864], lhs, rhs2,
                                    start=(c == 0), stop=(c == 1))
                            # evict + leaky relu into interleaved S:
                            # S col = 72*xw + 9*r + dy.
                            # DVE takes dys 0..4 (custom one-pass lrelu),
                            # ACT takes dys 5..8 (Prelu) -> balanced.
                            psb = ps[:, :]
                            PRL = psb.ap[0][0]
                            so = Sb.offset + 9 * r
                            dst1 = bass_rust.AP(
                                Sb.tensor, so,
                                [[RL, WH], [1, 5], [SW, WPAD]])
                            src1 = bass_rust.AP(
                                psb.tensor, psb.offset,
                                [[PRL, WH], [WPAD, 5], [1, WPAD]])
                            nc.vector._custom_dve(lrelu_op, out=dst1,
                                                  in0=src1, imm2=neg)
                            dst2 = bass_rust.AP(
                                Sb.tensor, so + 5,
                                [[RL, WH], [1, 4], [SW, WPAD]])
                            src2 = bass_rust.AP(
                                psb.tensor, psb.offset + 512,
                                [[PRL, WH], [WPAD, 4], [1, WPAD]])
                            nc.scalar.activation(dst2, src2, act_fn,
                                                 alpha=neg)

                        # band DMA: partition m reads contiguous
                        # S[m, 72m : 72m + 648]
                        diag = bass_rust.AP(Sb.tensor, Sb.offset,
                                            [[RL + SW, WH], [1, YB * NPATCH]])
                        nc.gpsimd.dma_start(out_ext[b, v, h, yb], diag)
    nc.finalize()
    return nc


_CACHE: dict = {}


def _get_nc() -> bass.Bass:
    if "nc" not in _CACHE:
        _CACHE["nc"] = build_nc(leaky=True)
    return _CACHE["nc"]


def _prep_inputs(feat1: np.ndarray, feat2: np.ndarray):
    f1 = np.ascontiguousarray(feat1, dtype=np.float32).astype(ml_dtypes.bfloat16)
    f2b = np.ascontiguousarray(feat2, dtype=np.float32).astype(ml_dtypes.bfloat16)
    f2 = np.zeros((B, C, HPAD, W2), dtype=ml_dtypes.bfloat16)
    f2[:, :, 4:4 + H, 4:4 + W] = f2b
    return f1, f2


def _assemble(core_outs: list) -> np.ndarray:
    # device layout: [b, v, h, yb, m(80), dx(9), r(8), dy(9)]
    # reference:     [b, 9*dy + dx, 48*v + 8*yb + r, 80*h + m]
    full = np.empty((B, NPATCH, H, W), dtype=np.float32)
    for i, a in enumerate(core_outs):
        a = np.asarray(a).reshape(
            NB, 2, 2, HU // YB, WH, 9, YB, 9).astype(np.float32)
        full[NB * i:NB * (i + 1)] = (
            a.transpose(0, 7, 5, 1, 3, 6, 2, 4).reshape(NB, NPATCH, H, W))
    return full


def kernel(feat1: np.ndarray, feat2: np.ndarray, **_ignored) -> np.ndarray:
    f1, f2 = _prep_inputs(np.asarray(feat1), np.asarray(feat2))
    nc = _get_nc()
    in_maps = [
        {"feat1": f1[NB * i:NB * (i + 1)], "feat2": f2[NB * i:NB * (i + 1)]}
        for i in range(NCORES)
    ]
    res = run_bass_kernel_spmd(nc, in_maps, list(range(NCORES)))
    return _assemble([res.results[i]["out"] for i in range(NCORES)])


# revision 5
# speedup vs baseline: 5.0011x; 5.0011x over previous
"""Correlation cost-volume (SpatialCorrelationSampler k=1, patch=9) + leaky ReLU.

Full inputs: feat1, feat2 [16, 256, 96, 160] f32.  Output [16, 81, 96, 160] f32.
corr[b, 9*i+j, y, x] = leaky_relu(sum_c f1[b,c,y,x] * f2[b,c,y+i-4,x+j-4], 0.1)

Strategy (8 NeuronCores, data-parallel over batch, 2 images/core), 2D-blocked:
  - host pre-casts both features to bf16, pre-pads feat2 to [C, 104, 168],
    and rearranges feat1 block-contiguous so every matmul lhsT is a single
    contiguous 128-column run (the Matmult stationary operand allows only
    one free dim).  All SBUF loads are single large contiguous DMAs.
  - blocks of M = 8 rows x 16 cols = 128 output positions per matmul:
      lhsT = f1 block                         [K=128, M=128]  (contiguous)
      rhs  = f2pad rows yy..yy+15, 24 cols    [K=128, N=16*24=384]
    so one matmul per (block, chunk) computes every (position, displacement)
    product the block needs; 2 C-chunks accumulate in one PSUM bank.
    96/8 = 12 y-blocks and 160/16 = 10 x-blocks tile the image exactly.
  - ScalarE/VectorE alternate evicting blocks PSUM -> SBUF with fused
    leaky-relu as plain contiguous copies (full engine rate), bf16.
  - one ~1MB DMA per (unit, y-block) ships the dense windows to DRAM;
    the host extracts the 81-displacement band per position with a free
    numpy as_strided view and upcasts to f32.
  vs the row-band v1: ~2x less HBM traffic (bf16 + denser windows), ~3x
  fewer TensorE columns, and eviction at full rate instead of 2B-scattered
  writes (the v1/v2 bottleneck: ~0.9-1.2us per eviction instruction).
"""

import numpy as np
import ml_dtypes

import bass_rust
import concourse.bacc as bacc
import concourse.bass as bass
import concourse.mybir as mybir
import concourse.tile as tile
from concourse.bass_utils import run_bass_kernel_spmd

B, C, H, W = 16, 256, 96, 160
NCORES = 8
NB = B // NCORES          # images per core
NPATCH = 81
HPAD = H + 8              # host-padded feat2 rows (104)
W2 = W + 8                # host-padded feat2 cols (168)
HU = H // 2               # rows per vertical-half unit (48)
HU2 = HU + 8              # padded rows held per unit (56)
MY, MX = 8, 16            # block geometry: MY*MX = 128 output positions
NYB = HU // MY            # y-blocks per unit (6)
NYBI = H // MY            # y-blocks per image (12)
NXB = W // MX             # x-blocks (10)
WY, WX = MY + 8, MX + 8   # rhs window (16 x 24)
NW = WY * WX              # window elems per block (384)

BF16 = mybir.dt.bfloat16


_LRELU_OP = None


def _get_lrelu_op():
    """Register a one-pass leaky-relu custom DVE op: out = max(x, x*imm2)."""
    global _LRELU_OP
    if _LRELU_OP is not None:
        return _LRELU_OP
    from concourse import dve_ops as dops
    from concourse.dve_spec import Spec, Src0, C2, maxx, lower
    from concourse.dve_uop import DveOpSpec
    name = "LRELU_ANT"
    if name in dops.CUSTOM_DVE_SPECS:
        _LRELU_OP = next(o for o in dops.OPS if o.name == name)
        return _LRELU_OP
    spec = Spec(
        body=maxx(Src0, Src0 * C2),
        reference=lambda in0, in1, c0, c1, c2: np.maximum(in0, in0 * c2))
    opcode = dops._CUSTOM_DVE_ROW_BASE + len(dops.OPS)
    shas = {}
    for ver in ("v3", "v4"):
        try:
            o = DveOpSpec(name=name, opcode=opcode, uops=lower(spec, ver=ver),
                          rd1_en=False)
            shas[ver] = o.sha(ver)
        except Exception:
            pass
    op = dops.DveOp(name, spec, subdim=False, uops_sha=shas)
    dops.OPS.append(op)
    dops.CUSTOM_DVE_SPECS[name] = spec
    dops._SUB_OPCODE_FOR_NAME[name] = opcode
    _LRELU_OP = op
    return op


def build_nc(leaky: bool = True, units: list | None = None,
             s_bufs: int = 3, ps_blocks: int = 2) -> bass.Bass:
    lrelu_op = _get_lrelu_op()
    nc = bacc.Bacc()
    f1_ext = nc.declare_dram_parameter(
        "feat1", [NB, C, H, W], BF16, isOutput=False)
    f2_ext = nc.declare_dram_parameter(
        "feat2", [NB, C, HPAD, W2], BF16, isOutput=False)
    out_ext = nc.declare_dram_parameter(
        "out", [NB, 2, NYB, 128, NXB * NW], BF16, isOutput=True)

    act_fn = (mybir.ActivationFunctionType.Prelu if leaky
              else mybir.ActivationFunctionType.Relu)
    neg = 0.1 if leaky else 0.0
    if units is None:
        units = [(b, v) for b in range(NB) for v in range(2)]

    with tile.TileContext(nc) as tc:
        with (
            tc.tile_pool(name="feat", bufs=2) as featp,
            tc.tile_pool(name="spool", bufs=s_bufs) as spool,
            tc.tile_pool(name="psum", bufs=8 // ps_blocks,
                         space="PSUM") as psump,
        ):
            for (b, v) in units:
                y0 = HU * v
                f1u = [featp.tile([128, HU * W], BF16,
                                  tag=f"f1u{c}", name=f"f1u{c}_{b}_{v}")
                       for c in range(2)]
                f2u = [featp.tile([128, HU2 * W2], BF16,
                                  tag=f"f2u{c}", name=f"f2u{c}_{b}_{v}")
                       for c in range(2)]
                for c in range(2):
                    nc.sync.dma_start(
                        f2u[c][:, :],
                        f2_ext[b, 128 * c:128 * (c + 1), y0:y0 + HU2, :])
                    nc.sync.dma_start(
                        f1u[c][:, :],
                        f1_ext[b, 128 * c:128 * (c + 1), y0:y0 + HU, :])

                for ybk in range(NYB):
                    yy0 = MY * ybk
                    S = spool.tile([128, NXB * NW], BF16, tag="S",
                                   name=f"S_{b}_{v}_{ybk}")
                    Sb = S[:, :]
                    RL = Sb.ap[0][0]
                    for xg in range(NXB // ps_blocks):
                        ps = psump.tile([128, 512 * ps_blocks],
                                        mybir.dt.float32, tag="ps",
                                        name=f"ps_{b}_{v}_{ybk}_{xg}")
                        psb = ps[:, :]
                        PRL = psb.ap[0][0]
                        for j in range(ps_blocks):
                            xb = xg * ps_blocks + j
                            x0 = MX * xb
                            for c in range(2):
                                f1b_ = f1u[c][:, :]
                                F1RL = f1b_.ap[0][0]
                                f2b_ = f2u[c][:, :]
                                F2RL = f2b_.ap[0][0]
                                lhs = bass_rust.AP(
                                    f1b_.tensor,
                                    f1b_.offset + (ybk * NXB + xb) * 128,
                                    [[F1RL, 128], [1, 128]])
                                rhs = bass_rust.AP(
                                    f2b_.tensor,
                                    f2b_.offset + yy0 * W2 + x0,
                                    [[F2RL, 128], [W2, WY], [1, WX]])
                                nc.tensor.matmul(
                                    ps[0:128, 512 * j:512 * j + NW],
                                    lhs, rhs,
                                    start=(c == 0), stop=(c == 1))
                            # contiguous eviction + leaky-relu, engines
                            # alternating by x-block parity
                            dst = bass_rust.AP(
                                Sb.tensor, Sb.offset + NW * xb,
                                [[RL, 128], [1, NW]])
                            src = bass_rust.AP(
                                psb.tensor, psb.offset + 512 * j,
                                [[PRL, 128], [1, NW]])
                            if xb % 2 == 0:
                                nc.vector._custom_dve(lrelu_op, out=dst,
                                                      in0=src, imm2=neg)
                            else:
                                nc.scalar.activation(dst, src, act_fn,
                                                     alpha=neg)

                    nc.gpsimd.dma_start(out_ext[b, v, ybk], Sb)
    nc.finalize()
    return nc


_CACHE: dict = {}


def _get_nc() -> bass.Bass:
    if "nc" not in _CACHE:
        _CACHE["nc"] = build_nc(leaky=True)
    return _CACHE["nc"]


def _prep_inputs(feat1: np.ndarray, feat2: np.ndarray):
    f1 = np.ascontiguousarray(feat1, dtype=np.float32).astype(ml_dtypes.bfloat16)
    # block-contiguous rearrange: [B,C,(ybk,yl),(xb,xl)] -> [B,C,ybk,xb,yl,xl]
    f1 = np.ascontiguousarray(
        f1.reshape(B, C, NYBI, MY, NXB, MX).transpose(0, 1, 2, 4, 3, 5)
        .reshape(B, C, H, W))
    f2b = np.ascontiguousarray(feat2, dtype=np.float32).astype(ml_dtypes.bfloat16)
    f2 = np.zeros((B, C, HPAD, W2), dtype=ml_dtypes.bfloat16)
    f2[:, :, 4:4 + H, 4:4 + W] = f2b
    return f1, f2


def _assemble(core_outs: list) -> np.ndarray:
    # device layout: [b, v, ybk, m=(yl,xl), xb, (yw, xw)] dense windows;
    # band element (yl, xl, dy, dx) lives at m = 16*yl + xl,
    # n = 24*(yl+dy) + (xl+dx).  Extract with a zero-copy strided view.
    full = np.empty((B, NPATCH, H, W), dtype=np.float32)
    for i, a in enumerate(core_outs):
        a = np.asarray(a).astype(np.float32)       # [NB,2,NYB,128,NXB*NW]
        a = np.ascontiguousarray(a)
        st = np.array(a.strides) // a.itemsize     # element strides
        s_b, s_v, s_ybk, s_m, s_n = st
        band = np.lib.stride_tricks.as_strided(
            a,
            shape=(NB, 2, NYB, MY, MX, NXB, 9, 9),
            strides=np.array([
                s_b, s_v, s_ybk,
                MX * s_m + WX,      # yl: next lhs row + next window row
                s_m + 1,            # xl: next lhs col + next window col
                NW,                 # xb
                WX,                 # dy
                1,                  # dx
            ]) * a.itemsize)
        # -> [b, dy, dx, v, ybk, yl, xb, xl] = [b, 81, 96, 160]
        full[NB * i:NB * (i + 1)] = (
            band.transpose(0, 6, 7, 1, 2, 3, 5, 4)
            .reshape(NB, NPATCH, H, W))
    return full


def kernel(feat1: np.ndarray, feat2: np.ndarray, **_ignored) -> np.ndarray:
    f1, f2 = _prep_inputs(np.asarray(feat1), np.asarray(feat2))
    nc = _get_nc()
    in_maps = [
        {"feat1": f1[NB * i:NB * (i + 1)], "feat2": f2[NB * i:NB * (i + 1)]}
        for i in range(NCORES)
    ]
    res = run_bass_kernel_spmd(nc, in_maps, list(range(NCORES)))
    return _assemble([res.results[i]["out"] for i in range(NCORES)])
